# revision 2
# baseline (speedup 1.0000x reference)
"""KAN layer Trainium2 kernel.

Math: uniform-knot cubic B-spline closed form.
  s = x/h - g0/h, w_t = s - t - 2, u = |w_t|
  am = min(u-2, 0), bm = min(am+1, 0)   (am = -relu(2-u), bm = -relu(1-u))
  B_t = -am^3/6 + (2/3) bm^3
  y[b,i,o]      = sum_t B_t[b,i] coef[i,o,t]            (postspline, transposed)
  postacts      = scale_sp*y + scale_base*silu(x)       (transposed)
  out[b,o]      = sum_i postacts[b,o,i]
  preacts[b,o,i]= x[b,i]

Device layout: T-layout basis tiles [(gl,il,t)=128 part, batch free], einsum and
all transposes as fp32r matmuls with host-built coefficient matrices; outputs
assembled [batch part, (o,i) free] and DMA'd contiguously.
"""

import numpy as np

import concourse.bass as bass
import concourse.bacc as bacc
import concourse.tile as tile
from concourse import mybir
from concourse.bass_utils import run_bass_kernel_spmd

N_CORES = 8
BATCH = 65536
BS = BATCH // N_CORES  # 8192 per core
IN, OUT, T = 64, 16, 8
NSC = BS // 512  # superchunks of 512
F32 = mybir.dt.float32
F32R = mybir.dt.float32r
ALU = mybir.AluOpType
AF = mybir.ActivationFunctionType

_CACHED_NC = None


def _r(ap):
    return ap.bitcast(F32R)


def build_nc():
    nc = bacc.Bacc()
    x = nc.dram_tensor("x", [BS, IN], F32, kind="ExternalInput")
    E_all = nc.dram_tensor("E_all", [64, 4, 128], F32, kind="ExternalInput")
    bias_all = nc.dram_tensor("bias_all", [128, 4], F32, kind="ExternalInput")
    Ca_all = nc.dram_tensor("Ca_all", [128, 4, 256], F32R, kind="ExternalInput")
    Cb_all = nc.dram_tensor("Cb_all", [128, 4, 256], F32R, kind="ExternalInput")
    Cpa_all = nc.dram_tensor("Cpa_all", [128, 4, 256], F32R, kind="ExternalInput")
    Cpb_all = nc.dram_tensor("Cpb_all", [128, 4, 256], F32R, kind="ExternalInput")
    Esc_all = nc.dram_tensor("Esc_all", [64, 4, 256], F32R, kind="ExternalInput")
    Coa_all = nc.dram_tensor("Coa_all", [128, 4, 16], F32R, kind="ExternalInput")
    Cob_all = nc.dram_tensor("Cob_all", [128, 4, 16], F32R, kind="ExternalInput")
    Sb = nc.dram_tensor("Sb", [64, 16], F32R, kind="ExternalInput")
    ident = nc.dram_tensor("ident", [128, 128], F32, kind="ExternalInput")

    o_out = nc.dram_tensor("o_out", [BS, OUT], F32, kind="ExternalOutput")
    o_pre = nc.dram_tensor("o_pre", [BS, 1024], F32, kind="ExternalOutput")
    o_pa = nc.dram_tensor("o_pa", [BS, 1024], F32, kind="ExternalOutput")
    o_ps = nc.dram_tensor("o_ps", [BS, 1024], F32, kind="ExternalOutput")

    with tile.TileContext(nc) as tc:
        with (
            tc.tile_pool(name="consts", bufs=1) as cp,
            tc.tile_pool(name="xin", bufs=3) as xp,
            tc.tile_pool(name="tsb", bufs=2) as tp,
            tc.tile_pool(name="ew", bufs=3) as ep,
            tc.tile_pool(name="b3", bufs=6) as bp,
            tc.tile_pool(name="osb", bufs=3) as op,
            tc.tile_pool(name="ps_xt", bufs=1, space="PSUM") as pxt,
            tc.tile_pool(name="ps_xr", bufs=2, space="PSUM") as pxr,
            tc.tile_pool(name="ps_ps", bufs=2, space="PSUM") as pps,
            tc.tile_pool(name="ps_pa", bufs=2, space="PSUM") as ppa,
            tc.tile_pool(name="ps_out", bufs=1, space="PSUM") as pou,
        ):
            E_sb = cp.tile([64, 4, 128], F32)
            nc.sync.dma_start(E_sb[:], E_all[:])
            bias_sb = cp.tile([128, 4], F32)
            nc.sync.dma_start(bias_sb[:], bias_all[:])
            Ca_sb = cp.tile([128, 4, 256], F32R)
            nc.sync.dma_start(Ca_sb[:], Ca_all[:])
            Cb_sb = cp.tile([128, 4, 256], F32R)
            nc.sync.dma_start(Cb_sb[:], Cb_all[:])
            Cpa_sb = cp.tile([128, 4, 256], F32R)
            nc.sync.dma_start(Cpa_sb[:], Cpa_all[:])
            Cpb_sb = cp.tile([128, 4, 256], F32R)
            nc.sync.dma_start(Cpb_sb[:], Cpb_all[:])
            Esc_sb = cp.tile([64, 4, 256], F32R)
            nc.sync.dma_start(Esc_sb[:], Esc_all[:])
            Coa_sb = cp.tile([128, 4, 16], F32R)
            nc.sync.dma_start(Coa_sb[:], Coa_all[:])
            Cob_sb = cp.tile([128, 4, 16], F32R)
            nc.sync.dma_start(Cob_sb[:], Cob_all[:])
            Sb_sb = cp.tile([64, 16], F32R)
            nc.sync.dma_start(Sb_sb[:], Sb[:])
            id_sb = cp.tile([128, 128], F32)
            nc.sync.dma_start(id_sb[:], ident[:])

            for sc in range(NSC):
                b0 = sc * 512
                x_sc = xp.tile([128, 4, 64], F32, tag="xsc")
                nc.sync.dma_start(
                    x_sc[:],
                    x.rearrange("(s c p) i -> s c p i", s=NSC, c=4)[sc].rearrange(
                        "c p i -> p c i"
                    ),
                )
                # transpose x -> xT [64, 512] via PE
                xt_ps = pxt.tile([64, 512], F32, tag="xt")
                for c in range(4):
                    nc.tensor.transpose(
                        xt_ps[:, c * 128 : (c + 1) * 128], x_sc[:, c, :], id_sb[:]
                    )
                xt_sb = tp.tile([64, 512], F32, tag="xt_sb")
                nc.scalar.activation(xt_sb[:], xt_ps[:], AF.Copy)
                silu_sb = tp.tile([64, 512], F32R, tag="silu")
                nc.scalar.activation(silu_sb[:], xt_sb[:], AF.Silu)

                am3s, bm3s = [], []
                for tg in range(4):
                    xr_ps = pxr.tile([128, 512], F32, tag="xr")
                    nc.tensor.matmul(
                        xr_ps[:], E_sb[:, tg, :], xt_sb[:], start=True, stop=True
                    )
                    u = ep.tile([128, 512], F32, tag="u")
                    nc.scalar.activation(u[:], xr_ps[:], AF.Abs, bias=bias_sb[:, tg : tg + 1])
                    am = ep.tile([128, 512], F32, tag="am")
                    nc.vector.tensor_scalar(am[:], u[:], 2.0, 0.0, ALU.subtract, ALU.min)
                    bm = ep.tile([128, 512], F32, tag="bm")
                    nc.vector.tensor_scalar(bm[:], am[:], 1.0, 0.0, ALU.add, ALU.min)
                    a2 = ep.tile([128, 512], F32, tag="a2")
                    nc.scalar.activation(a2[:], am[:], AF.Square)
                    b2 = ep.tile([128, 512], F32, tag="b2")
                    nc.scalar.activation(b2[:], bm[:], AF.Square)
                    am3 = bp.tile([128, 512], F32R, tag="am3")
                    nc.vector.tensor_tensor(am3[:], a2[:], am[:], ALU.mult)
                    bm3 = bp.tile([128, 512], F32R, tag="bm3")
                    nc.vector.tensor_tensor(bm3[:], b2[:], bm[:], ALU.mult)
                    am3s.append(am3)
                    bm3s.append(bm3)

                for s in range(4):
                    sl = slice(s * 128, (s + 1) * 128)
                    # ---- out (bs,16)
                    out_ps = pou.tile([128, 16], F32, tag="out")
                    for tg in range(4):
                        nc.tensor.matmul(
                            out_ps[:], am3s[tg][:, sl], Coa_sb[:, tg, :],
                            start=(tg == 0), stop=False,
                        )
                        nc.tensor.matmul(
                            out_ps[:], bm3s[tg][:, sl], Cob_sb[:, tg, :],
                            start=False, stop=False,
                        )
                    nc.tensor.matmul(
                        out_ps[:], silu_sb[:, sl], Sb_sb[:], start=False, stop=True
                    )
                    out_sb = op.tile([128, 16], F32, tag="outsb")
                    nc.vector.tensor_copy(out_sb[:], out_ps[:])
                    nc.sync.dma_start(o_out[b0 + s * 128 : b0 + (s + 1) * 128, :], out_sb[:])

                    # ---- postspline / postacts
                    psl = op.tile([128, 16, 64], F32, tag="psl")
                    pal = op.tile([128, 16, 64], F32, tag="pal")
                    for tg in range(4):
                        ps_ps = pps.tile([128, 256], F32, tag="ps")
                        nc.tensor.matmul(
                            ps_ps[:], am3s[tg][:, sl], Ca_sb[:, tg, :],
                            start=True, stop=False,
                        )
                        nc.tensor.matmul(
                            ps_ps[:], bm3s[tg][:, sl], Cb_sb[:, tg, :],
                            start=False, stop=True,
                        )
                        # cols (gl,o,il) -> dest [o, 16*tg+8*gl+il]
                        dst = bass.AP(
                            tensor=psl.tensor,
                            offset=psl.offset + tg * 16,
                            ap=[psl.ap[0], [8, 2], [64, 16], [1, 8]],
                        )
                        src = ps_ps.rearrange("p (gl o il) -> p gl o il", gl=2, o=16)
                        nc.scalar.activation(dst, src, AF.Copy)

                        pa_ps = ppa.tile([128, 256], F32, tag="pa")
                        nc.tensor.matmul(
                            pa_ps[:], silu_sb[:, sl], Esc_sb[:, tg, :],
                            start=True, stop=False,
                        )
                        nc.tensor.matmul(
                            pa_ps[:], am3s[tg][:, sl], Cpa_sb[:, tg, :],
                            start=False, stop=False,
                        )
                        nc.tensor.matmul(
                            pa_ps[:], bm3s[tg][:, sl], Cpb_sb[:, tg, :],
                            start=False, stop=True,
                        )
                        dst2 = bass.AP(
                            tensor=pal.tensor,
                            offset=pal.offset + tg * 16,
                            ap=[pal.ap[0], [8, 2], [64, 16], [1, 8]],
                        )
                        src2 = pa_ps.rearrange("p (gl o il) -> p gl o il", gl=2, o=16)
                        nc.vector.tensor_copy(dst2, src2)
                    row = slice(b0 + s * 128, b0 + (s + 1) * 128)
                    nc.sync.dma_start(o_ps[row, :], psl.rearrange("p o i -> p (o i)"))
                    nc.sync.dma_start(o_pa[row, :], pal.rearrange("p o i -> p (o i)"))
                    # ---- preacts: broadcast x along o via DMA
                    xsl = x_sc[:, s, :]
                    bcast = bass.AP(
                        tensor=xsl.tensor,
                        offset=xsl.offset,
                        ap=[xsl.ap[0], [0, 16], xsl.ap[-1]],
                    )
                    nc.gpsimd.dma_start(o_pre[row, :], bcast)

    nc.finalize()
    return nc


def _build_consts(grid, coef, scale_base, scale_sp):
    g0 = grid[:, 0].astype(np.float64)
    h = (grid[:, 1] - grid[:, 0]).astype(np.float64)
    inv_h = (1.0 / h).astype(np.float32)
    E_all = np.zeros((64, 4, 128), np.float32)
    bias_all = np.zeros((128, 4), np.float32)
    Ca = np.zeros((128, 4, 256), np.float32)
    Cb = np.zeros((128, 4, 256), np.float32)
    Cpa = np.zeros((128, 4, 256), np.float32)
    Cpb = np.zeros((128, 4, 256), np.float32)
    Esc = np.zeros((64, 4, 256), np.float32)
    Coa = np.zeros((128, 4, 16), np.float32)
    Cob = np.zeros((128, 4, 16), np.float32)
    for tg in range(4):
        for gl in range(2):
            for il in range(8):
                i = (2 * tg + gl) * 8 + il
                for t in range(T):
                    p = gl * 64 + il * 8 + t
                    E_all[i, tg, p] = inv_h[i]
                    bias_all[p, tg] = np.float32(-(g0[i] / h[i]) - (t + 2))
                    for o in range(OUT):
                        col = gl * 128 + o * 8 + il
                        c = coef[i, o, t]
                        Ca[p, tg, col] = -c / 6.0
                        Cb[p, tg, col] = (2.0 / 3.0) * c
                        Cpa[p, tg, col] = -c * scale_sp[i, o] / 6.0
                        Cpb[p, tg, col] = (2.0 / 3.0) * c * scale_sp[i, o]
                        Coa[p, tg, o] = -c * scale_sp[i, o] / 6.0
                        Cob[p, tg, o] = (2.0 / 3.0) * c * scale_sp[i, o]
                for o in range(OUT):
                    Esc[i, tg, gl * 128 + o * 8 + il] = scale_base[i, o]
    return dict(
        E_all=E_all, bias_all=bias_all, Ca_all=Ca, Cb_all=Cb, Cpa_all=Cpa,
        Cpb_all=Cpb, Esc_all=Esc, Coa_all=Coa, Cob_all=Cob,
        Sb=scale_base.astype(np.float32), ident=np.eye(128, dtype=np.float32),
    )


def kernel(x, grid, coef, scale_base, scale_sp):
    global _CACHED_NC
    x = np.ascontiguousarray(x, np.float32)
    consts = _build_consts(
        np.asarray(grid, np.float32), np.asarray(coef, np.float32),
        np.asarray(scale_base, np.float32), np.asarray(scale_sp, np.float32),
    )
    if _CACHED_NC is None:
        _CACHED_NC = build_nc()
    nc = _CACHED_NC
    xs = x.reshape(N_CORES, BS, IN)
    in_maps = [dict(consts, x=np.ascontiguousarray(xs[c])) for c in range(N_CORES)]
    res = run_bass_kernel_spmd(nc, in_maps, core_ids=list(range(N_CORES)))
    out = np.concatenate([r["o_out"] for r in res.results], axis=0)
    pre = np.concatenate([r["o_pre"] for r in res.results], axis=0).reshape(BATCH, OUT, IN)
    pa = np.concatenate([r["o_pa"] for r in res.results], axis=0).reshape(BATCH, OUT, IN)
    ps = np.concatenate([r["o_ps"] for r in res.results], axis=0).reshape(BATCH, OUT, IN)
    return out, pre, pa, ps
